# revision 56
# baseline (speedup 1.0000x reference)
"""Trainium2 Bass kernel for nn_AttnBlock (B=1, C=128, H=32, W=128, 8 heads).

Sharding: one attention head per NeuronCore (8 heads / 8 cores). Each core
computes its head's attention over L=4096 positions and the final W-axis
projection for its 16-channel output slab. Host gathers 8 slabs.

v5 = v2 baseline + faster startup + schedule tuning (105.6us -> 103.4us):
  * ACT exp-table warm emitted before any DMA so the Pool-queue memset isn't
    stuck behind SWDGE descriptor generation.
  * chunks 0-1 compute S^T in bf16 straight from x_sb/g_bf: no fp8
    partition-pair permute DMA on the startup critical path (the 0.25
    x-scale is folded into the exp affine for those chunks).  chunks 2-7
    use the fp8 DoubleRow path as in v2.
  * input DMAs: x0 + cb + x1 on the sync (HWDGE) queue, gw and the bulk
    merged into large transfers on the Pool (SWDGE) queue.
  * AV emission lag 5 pairs; per-chunk epilogue evacuation one slot later
    (gi5) so its last-AV dependency never parks at the ACT FIFO head;
    final-chunk evac/osb unsplit (the split's second half gated the
    transposes later than one full copy).
  (Measured dead ends, for future sessions: fused FD>=2048 exp ops are
  impossible -- matmul writes into slices of ONE psum tensor serialize
  ~2x in the tile dep-tracker, and separate tiles can't share an AP, so
  exp stays FD-1024 via the 3-buf squad-pool rotation.  GPSIMD/DMA can't
  read PSUM, so ACT+DVE at ~1 elem/cycle/partition each are a hard
  ~82us busy floor for the 16.7M-score exp per core.)

v2 design recap:
  * S^T via the rank-16 factor-through-weights trick: G = (4*Wk^T Wq) @ x,
    then every S^T tile is x_tile^T @ G_chunk.
  * q-bias folded EXACTLY into a per-key reweighting of V: w_j =
    exp(4*bq.k_j), produced as a 17th column of the v matmul.
  * exp tiles (128 x 1024) split between ScalarE (exact exp -> fp8e4) and
    VectorE (Schraudolph int-bit trick -> uint8 bits).
  * A@V in fp8e4 DoubleRow with the softmax denominator as a 17th weight
    column; epilogue: idmatmul transpose, reciprocal normalize, bf16
    W-axis projection.
"""

import math as _math

import numpy as np

N_CORES = 8
C = 128
H = 32
W = 128
L = H * W  # 4096
F = 8  # heads
D = 16  # head dim
CHUNK = 512
NCHUNK = L // CHUNK  # 8
NJT = L // 128  # 32 j-tiles
SHIFT = 2.5  # global exp shift for fp8 range (cancels in softmax)
A8 = 8.0 / _math.log(2.0)  # Schraudolph scale for e4m3 bits
B8P = (56.0 - 0.5) - SHIFT * A8  # magic + shift folded
CB_W = 20  # f32 cblob: idpad (17,18) | negshift col
BB_W = 976

# cost-model constants for build-time ACT/DVE load balancing (ns)
_ACT_CY = 1e9 / 1.2e9
_DVE_CY = 1e9 / 0.96e9


def _act_cost(fd):
    return (fd + 222) * _ACT_CY


def _dve_cost(fd, psum=True):
    return (fd + (120 if psum else 58)) * _DVE_CY


_CACHE = {}


def _build():
    import concourse.tile as tile
    from concourse import bacc, mybir

    f32 = mybir.dt.float32
    bf16 = mybir.dt.bfloat16
    fp8 = mybir.dt.float8e4
    u8 = mybir.dt.uint8
    Exp = mybir.ActivationFunctionType.Exp
    DR = mybir.MatmulPerfMode.DoubleRow

    nc = bacc.Bacc("TRN2", target_bir_lowering=False, debug=False)

    x_d = nc.dram_tensor("x_cl", [C, L], bf16, kind="ExternalInput").ap()
    x8_d = nc.dram_tensor("x8", [C // 2, 2, L], fp8, kind="ExternalInput").ap()
    cb_d = nc.dram_tensor("cblob", [C, CB_W], f32, kind="ExternalInput").ap()
    bb_d = nc.dram_tensor("bblob", [C, BB_W], bf16, kind="ExternalInput").ap()
    gw_d = nc.dram_tensor("gwt", [C, 128], bf16, kind="ExternalInput").ap()
    out_d = nc.dram_tensor("out", [D, L], f32, kind="ExternalOutput").ap()

    # build-time engine load (ns) for balancing flexible work
    load = {"act": 1283.0, "dve": 0.0}  # act table load charged up front

    def pick_engine():
        return "act" if load["act"] <= load["dve"] else "dve"

    with tile.TileContext(nc) as tc:
        with (
            tc.tile_pool(name="consts", bufs=1) as consts,
            tc.tile_pool(name="accsb", bufs=2) as accsbp,
            tc.tile_pool(name="episb", bufs=4) as episb,
        ):
            cb = consts.tile([C, CB_W], f32)
            idpad = cb[0:17, 0:18]
            negshift = cb[:, 18:19]
            bb = consts.tile([C, BB_W], bf16)
            wpbf = bb[:, 0:128]
            wvb = bb[:, 128:160]
            gw_t = consts.tile([C, 128], bf16)
            gw = gw_t[:]
            bp512 = bb[0:1, 288:800]
            ones16 = bb[0:1, 800:816]
            bv32 = bb[0:1, 816:848]
            ones128row = bb[0:1, 848:976]

            x_sb = consts.tile([C, L], bf16)
            x8_sb = consts.tile([C // 2, 2, L], fp8)
            g_bf = consts.tile([C, 2, CHUNK], bf16)  # G chunks 0-1 (bf16 path)
            g_f8 = consts.tile([C, 6 * CHUNK], fp8)  # G chunks 2-7
            g8_sb = consts.tile([C // 2, 2, 6 * CHUNK], fp8)
            et = consts.tile([C, NJT, CHUNK], fp8)
            v_sb = consts.tile([C, NJT, 32], fp8)
            wexp = consts.tile([C, NJT], f32)

            # warm the ACT exp table immediately; emitted before the DMAs so
            # the Pool-queue memset isn't stuck behind SWDGE descriptor gen.
            dummy = episb.tile([1, 2], f32, tag="dummy")
            nc.gpsimd.memset(dummy[:], 0.5)
            nc.scalar.activation(out=dummy[:], in_=dummy[:], func=Exp)

            # --- input DMA: critical chain on sync (HWDGE), bulk on Pool ---
            nc.gpsimd.dma_start(out=gw_t, in_=gw_d)
            nc.sync.dma_start(out=x_sb[:, 0:512], in_=x_d[:, 0:512])
            nc.sync.dma_start(out=cb, in_=cb_d)
            nc.sync.dma_start(out=x_sb[:, 512:1024], in_=x_d[:, 512:1024])
            nc.gpsimd.dma_start(out=bb, in_=bb_d)
            nc.gpsimd.dma_start(out=x_sb[:, 1024:2560], in_=x_d[:, 1024:2560])
            nc.gpsimd.dma_start(out=x_sb[:, 2560:4096], in_=x_d[:, 2560:4096])
            nc.gpsimd.dma_start(out=x8_sb[:, :, 0:2048], in_=x8_d[:, :, 0:2048])
            nc.gpsimd.dma_start(out=x8_sb[:, :, 2048:4096], in_=x8_d[:, :, 2048:4096])

            with (
                tc.tile_pool(name="ps_s", bufs=3, space="PSUM") as ps_s,
                tc.tile_pool(name="ps_acc", bufs=1, space="PSUM") as ps_acc,
                tc.tile_pool(name="ps_epi", bufs=1, space="PSUM") as ps_epi,
            ):
                # ---- G helpers ----
                def emit_g_bf(t):
                    # G chunk t (0/1) in bf16 via a squad-pool slot; the first
                    # evacuation goes to DVE (ACT is still loading its table).
                    gps = ps_s.tile([C, 2, CHUNK], f32, tag="squad", name=f"gbf{t}")
                    nc.tensor.matmul(
                        gps[:, 0, :], gw, x_sb[:, t * 512 : (t + 1) * 512],
                        start=True, stop=True,
                    )
                    if t == 0:
                        nc.vector.tensor_copy(g_bf[:, t, :], gps[:, 0, :])
                        load["dve"] += _dve_cost(CHUNK)
                    else:
                        nc.scalar.copy(g_bf[:, t, :], gps[:, 0, :])
                        load["act"] += _act_cost(CHUNK)

                def emit_g8_slice(k):
                    # G chunks (2+2k, 3+2k) -> fp8 + partition-pair permute
                    gps = ps_s.tile([C, 2, CHUNK], f32, tag="squad", name=f"g8s{k}")
                    for t in range(2):
                        ch = 2 + 2 * k + t
                        nc.tensor.matmul(
                            gps[:, t, :], gw, x_sb[:, ch * 512 : (ch + 1) * 512],
                            start=True, stop=True,
                        )
                    sl = slice(1024 * k, 1024 * k + 1024)
                    nc.scalar.copy(g_f8[:, sl], gps[:])
                    load["act"] += _act_cost(2 * CHUNK)
                    for e in range(2):
                        nc.sync.dma_start(out=g8_sb[:, e, sl], in_=g_f8[e : C : 2, sl])

                def emit_v_group(g):
                    vps = ps_s.tile([C, 8, 32], f32, tag="squad", name=f"vps{g}")
                    for u in range(8):
                        t = 8 * g + u
                        nc.tensor.matmul(
                            vps[:, u, :], ones128row, bv32,
                            start=True, stop=False, skip_group_check=True,
                        )
                        nc.tensor.matmul(
                            vps[:, u, :], x_sb[:, t * 128 : (t + 1) * 128], wvb,
                            start=False, stop=True, skip_group_check=True,
                        )
                    nc.scalar.activation(
                        out=wexp[:, 8 * g : 8 * g + 8], in_=vps[:, :, 17], func=Exp
                    )
                    load["act"] += _act_cost(8)
                    nc.vector.tensor_tensor(
                        out=v_sb[:, 8 * g : 8 * g + 8, :],
                        in0=vps[:],
                        in1=wexp[:, 8 * g : 8 * g + 8].broadcast_to([C, 8, 32]),
                        op=mybir.AluOpType.mult,
                    )
                    load["dve"] += _dve_cost(256)

                # ---- main loop: flat global schedule, cross-boundary AV lag ----
                NG = 16  # groups (== DR pairs) per chunk

                def exp_consts(c):
                    # chunks 0-1 compute S^T at 4x scale (x not pre-scaled by
                    # 0.25); fold the 0.25 into the exp affine instead.
                    return (0.25, A8 * 0.25) if c < 2 else (1.0, A8)

                def emit_sts(c, gi, squad):
                    csl = slice(c * CHUNK, (c + 1) * CHUNK)
                    for t in range(2):
                        j = 2 * gi + t
                        if c < 2:
                            nc.tensor.matmul(
                                squad[:, t, :],
                                x_sb[:, j * 128 : (j + 1) * 128],
                                g_bf[:, c, :],
                                start=True, stop=True,
                            )
                        else:
                            nc.tensor.matmul(
                                squad[:, t, :],
                                x8_sb[:, :, j * 128 : (j + 1) * 128],
                                g8_sb[:, :, (c - 2) * CHUNK : (c - 1) * CHUNK],
                                start=True, stop=True,
                                perf_mode=DR,
                            )

                def emit_exp_split(c, gi, squad, ca=288):
                    # latency/balance: columns 0:ca on ACT, rest on DVE
                    scale, a8 = exp_consts(c)
                    j0 = 2 * gi
                    nc.scalar.activation(
                        out=et[:, j0 : j0 + 2, 0:ca],
                        in_=squad[:, :, 0:ca],
                        func=Exp,
                        bias=negshift,
                        scale=scale,
                    )
                    load["act"] += _act_cost(2 * ca)
                    nc.vector.tensor_scalar(
                        out=et[:, j0 : j0 + 2, ca:CHUNK].bitcast(u8),
                        in0=squad[:, :, ca:CHUNK],
                        scalar1=a8,
                        scalar2=B8P,
                        op0=mybir.AluOpType.mult,
                        op1=mybir.AluOpType.add,
                    )
                    load["dve"] += _dve_cost(2 * (CHUNK - ca))

                def emit_exp(c, gi, squad, parity=0, sliver=0):
                    scale, a8 = exp_consts(c)
                    j0 = 2 * gi
                    eng = "act" if parity == 0 else "dve"
                    if eng == "act":
                        nc.scalar.activation(
                            out=et[:, j0 : j0 + 2, :],
                            in_=squad[:],
                            func=Exp,
                            bias=negshift,
                            scale=scale,
                        )
                        load["act"] += _act_cost(2 * CHUNK)
                    else:
                        cw = CHUNK - sliver
                        nc.vector.tensor_scalar(
                            out=et[:, j0 : j0 + 2, 0:cw].bitcast(u8),
                            in0=squad[:, :, 0:cw],
                            scalar1=a8,
                            scalar2=B8P,
                            op0=mybir.AluOpType.mult,
                            op1=mybir.AluOpType.add,
                        )
                        load["dve"] += _dve_cost(2 * cw)
                        if sliver:
                            # rebalance: ACT picks up a column sliver of this
                            # DVE exp (DVE is the chunk-bound engine)
                            nc.scalar.activation(
                                out=et[:, j0 : j0 + 2, cw:CHUNK],
                                in_=squad[:, :, cw:CHUNK],
                                func=Exp,
                                bias=negshift,
                                scale=scale,
                            )
                            load["act"] += _act_cost(2 * sliver)

                def emit_av(acc_c, p):
                    nc.tensor.matmul(
                        acc_c[:],
                        v_sb[:, 2 * p : 2 * p + 2, 0:17],
                        et[:, 2 * p : 2 * p + 2, :],
                        start=(p == 0),
                        stop=(p == NG - 1),
                        perf_mode=DR,
                        skip_group_check=True,
                    )

                def emit_epi_evac(acc_c):
                    acc_sb = accsbp.tile([17, CHUNK], f32, tag="accsb")
                    nc.scalar.copy(acc_sb[:], acc_c[:])
                    load["act"] += _act_cost(CHUNK)
                    return acc_sb

                def emit_epi_norm(acc_sb):
                    tps4 = ps_s.tile([C, 4, 18], f32, tag="squad")
                    for s in range(4):
                        nc.tensor.matmul(
                            tps4[:, s, :],
                            acc_sb[:, s * 128 : (s + 1) * 128],
                            idpad,
                            start=True, stop=True,
                        )
                    recip4 = episb.tile([C, 4], f32, tag="recip")
                    nc.vector.reciprocal(recip4[:], tps4[:, :, 16])
                    load["dve"] += _dve_cost(4)
                    onorm4 = episb.tile([C, 4, 16], bf16, tag="onorm")
                    nc.vector.tensor_tensor(
                        out=onorm4[:],
                        in0=tps4[:, :, 0:16],
                        in1=recip4[:].broadcast_to([C, 4, 16]),
                        op=mybir.AluOpType.mult,
                    )
                    load["dve"] += _dve_cost(64)
                    return onorm4

                def emit_epi_proj(onorm4, c_prev):
                    pps = ps_epi.tile([D, CHUNK], f32, tag="pps")
                    nc.tensor.matmul(
                        pps[:], ones16, bp512,
                        start=True, stop=False, skip_group_check=True,
                    )
                    for s in range(4):
                        nc.tensor.matmul(
                            pps[:, s * 128 : (s + 1) * 128],
                            onorm4[:, s, :],
                            wpbf,
                            start=False, stop=(s == 3), skip_group_check=True,
                        )
                    osb = episb.tile([D, CHUNK], f32, tag="osb")
                    nc.scalar.copy(osb[:], pps[:])
                    load["act"] += _act_cost(CHUNK)
                    nc.sync.dma_start(
                        out=out_d[:, c_prev * CHUNK : (c_prev + 1) * CHUNK],
                        in_=osb[:],
                    )

                AV_LAG = 5
                EV_GI = 5
                PJ_GI = 9
                emit_g_bf(0)
                _pro = {
                    (0, 1): lambda: emit_g_bf(1),
                    (0, 3): lambda: emit_v_group(0),
                    (0, 5): lambda: emit_g8_slice(0),
                    (0, 7): lambda: emit_v_group(1),
                    (0, 9): lambda: emit_g8_slice(1),
                    (0, 11): lambda: (emit_g8_slice(2), emit_v_group(2)),
                    (0, 13): lambda: emit_v_group(3),
                }
                accs = {}
                epi = {}  # c -> dict of staged products
                av_next = 0  # global AV emission cursor (over 128 pairs)
                for G in range(NCHUNK * NG):
                    c, gi = divmod(G, NG)
                    squad = ps_s.tile([C, 2, CHUNK], f32, tag="squad")
                    emit_sts(c, gi, squad)
                    if c == NCHUNK - 1 and gi >= 14:
                        emit_exp_split(c, gi, squad)
                    else:
                        emit_exp(c, gi, squad, parity=G % 2)
                    if (c, gi) in _pro:
                        _pro.pop((c, gi))()
                    # drain AV pairs whose exp is AV_LAG groups back
                    while av_next <= G - AV_LAG:
                        cp, p = divmod(av_next, NG)
                        if p == 0:
                            accs[cp] = ps_acc.tile([17, CHUNK], f32, tag="acc", name=f"acc{cp}")
                        emit_av(accs[cp], p)
                        av_next += 1
                    if gi == EV_GI and c > 0:
                        # one slot after all of chunk c-1's AVs executed: the
                        # evacuation's acc dependency is already satisfied, so
                        # it never parks at the ACT FIFO head
                        epi[c - 1] = {"acc_sb": emit_epi_evac(accs.pop(c - 1))}
                    if gi == EV_GI + 1 and c > 0:
                        epi[c - 1]["onorm"] = emit_epi_norm(epi[c - 1]["acc_sb"])
                    if gi == PJ_GI and c > 0:
                        emit_epi_proj(epi.pop(c - 1)["onorm"], c - 1)
                while av_next < NCHUNK * NG:
                    cp, p = divmod(av_next, NG)
                    if p == 0:
                        accs[cp] = ps_acc.tile([17, CHUNK], f32, tag="acc", name=f"acc{cp}")
                    emit_av(accs[cp], p)
                    av_next += 1
                c_last = NCHUNK - 1
                acc_l = accs.pop(c_last)
                acc_sb = accsbp.tile([17, CHUNK], f32, tag="accsb")
                nc.scalar.copy(acc_sb[:], acc_l[:])
                onorm_l = emit_epi_norm(acc_sb)
                pps = ps_epi.tile([D, CHUNK], f32, tag="pps")
                nc.tensor.matmul(
                    pps[:], ones16, bp512,
                    start=True, stop=False, skip_group_check=True,
                )
                for s in range(4):
                    nc.tensor.matmul(
                        pps[:, s * 128 : (s + 1) * 128],
                        onorm_l[:, s, :],
                        wpbf,
                        start=False, stop=(s == 3), skip_group_check=True,
                    )
                osb = episb.tile([D, CHUNK], f32, tag="osb")
                nc.scalar.copy(osb[:], pps[:])
                nc.sync.dma_start(
                    out=out_d[:, c_last * CHUNK : (c_last + 1) * CHUNK],
                    in_=osb[:],
                )

    nc.compile()
    return nc


def _get_program():
    if "nc" not in _CACHE:
        _CACHE["nc"] = _build()
    return _CACHE["nc"]


def _make_in_maps(x, w_qkv, b_qkv, w_proj, b_proj):
    import ml_dtypes

    bf = ml_dtypes.bfloat16
    x_f32 = np.asarray(x, dtype=np.float32).reshape(C, L)
    x_cl = np.ascontiguousarray(x_f32.astype(bf))
    x8 = np.ascontiguousarray(
        (x_f32 * 0.25).reshape(C // 2, 2, L).astype(ml_dtypes.float8_e4m3)
    )
    w_qkv = np.asarray(w_qkv, dtype=np.float32)
    b_qkv = np.asarray(b_qkv, dtype=np.float32)
    w_proj = np.asarray(w_proj, dtype=np.float32)
    b_proj = np.asarray(b_proj, dtype=np.float32)
    wpT = np.ascontiguousarray(w_proj.T)

    cbase = np.zeros((C, CB_W), dtype=np.float32)
    cbase[0:17, 0:17] = np.eye(17, dtype=np.float32)  # idpad (col 17 zero)
    cbase[:, 18] = -SHIFT

    in_maps = []
    for i in range(N_CORES):
        rows_q = np.arange(D) * 24 + i * 3
        Wq = w_qkv[rows_q]
        Wk = w_qkv[rows_q + 1]
        Wv = w_qkv[rows_q + 2]
        bq = b_qkv[rows_q]
        bv = b_qkv[rows_q + 2]

        bbl = np.zeros((C, BB_W), dtype=bf)
        bbl[:, 0:128] = wpT.astype(bf)
        wvb = np.zeros((C, 32), dtype=np.float32)
        wvb[:, 0:16] = Wv.T
        wvb[:, 17] = 4.0 * (Wk.T @ bq)
        bbl[:, 128:160] = wvb.astype(bf)
        gwt = np.ascontiguousarray((16.0 * (Wq.T @ Wk)).astype(bf))
        bbl[0, 288:800] = np.tile(b_proj, 4).astype(bf)
        bbl[0, 800:816] = np.ones(16, dtype=bf)
        bv32 = np.zeros(32, dtype=np.float32)
        bv32[0:16] = bv
        bv32[16] = 1.0
        bbl[0, 816:848] = bv32.astype(bf)
        bbl[0, 848:976] = np.ones(128, dtype=bf)
        in_maps.append(
            {"x_cl": x_cl, "x8": x8, "cblob": cbase, "bblob": bbl, "gwt": gwt}
        )
    return in_maps


def _run(in_maps, trace=False):
    from concourse.bass_utils import run_bass_kernel_spmd

    nc = _get_program()
    return run_bass_kernel_spmd(nc, in_maps, list(range(N_CORES)), trace=trace)


def _assemble(results):
    out = np.empty((1, C, H, W), dtype=np.float32)
    for i in range(N_CORES):
        out[0, i * D : (i + 1) * D] = results[i]["out"].reshape(D, H, W)
    return out


def kernel(x, w_qkv, b_qkv, w_proj, b_proj):
    in_maps = _make_in_maps(x, w_qkv, b_qkv, w_proj, b_proj)
    r = _run(in_maps, trace=False)
    return _assemble(r.results)


def kernel_with_timing(x, w_qkv, b_qkv, w_proj, b_proj):
    """Like kernel() but also returns an HW execution time estimate in ns."""
    in_maps = _make_in_maps(x, w_qkv, b_qkv, w_proj, b_proj)
    try:
        r = _run(in_maps, trace=True)
        exec_ns = r.exec_time_ns
    except ModuleNotFoundError:
        r = _run(in_maps, trace=False)
        exec_ns = None
    if exec_ns is None:
        exec_ns = _CACHE.get("tlsim_ns")
        if exec_ns is None:
            from concourse.timeline_sim import TimelineSim

            exec_ns = int(TimelineSim(_get_program()).simulate())
            _CACHE["tlsim_ns"] = exec_ns
    return _assemble(r.results), exec_ns
